# revision 47
# baseline (speedup 1.0000x reference)
"""Causal multi-head self-attention on 8 Trainium2 NeuronCores.

Sharding: (batch, head-group).  Core c owns batch c//2 and heads
(c%2)*8 .. +8 (= 512 of the 1024 feature dims).  Each core:
  - projects Q^T/K^T (dim-major) and V (row-major) for its 512 dims,
  - runs causal attention for its 8 heads over its batch,
  - computes the partial output projection  attn_out @ wo_w[dims, :].
The host sums the 2 partial projections per batch and adds wo_b.

All matmuls run in bf16 (fp32 PSUM accumulation); fp32r was measured
executing as multi-pass fp32_mode=HIGH with heavy power throttling.
Scores are computed transposed (S^T[k, q]); an interleaved ones-column
per head in the V tile makes the P@V matmul emit softmax denominators.
Normalization uses reciprocal_approx_fast + gpsimd partition_broadcast
(no PE broadcast matmuls, no slow InstReciprocal).
Projection matmul groups are interleaved between attention units so the
PE stays busy while the Scalar engine streams the exps.
"""
import sys
sys.path.insert(0, "/opt/trn_rl_repo")

import numpy as np
import ml_dtypes
import concourse.bass as bass
import concourse.mybir as mybir
from concourse import bacc
from concourse.tile import TileContext
from concourse.bass_utils import run_bass_kernel_spmd

B, S, E = 4, 2048, 1024
H, D = 16, 64
NCORES = 8
HPC = 8                 # heads per core
NHC = 4                 # 128-dim chunks per core (2 heads each)
VW = D + 1              # v cols per head incl ones column
NSC = S // 128          # 16 k/s chunks of 128

f32 = mybir.dt.float32
bf16 = mybir.dt.bfloat16
AF = mybir.ActivationFunctionType
BF = ml_dtypes.bfloat16

_CACHE = {}


def build_nc(taps=False):
    nc = bacc.Bacc("TRN2", target_bir_lowering=False, debug=False)

    xT = nc.declare_dram_parameter("xT", [E, S], bf16, isOutput=False)
    wq = nc.declare_dram_parameter("wq", [E, 512], bf16, isOutput=False)
    wk = nc.declare_dram_parameter("wk", [E, 512], bf16, isOutput=False)
    wv = nc.declare_dram_parameter("wv", [E, 512], bf16, isOutput=False)
    wo = nc.declare_dram_parameter("wo", [512, E], bf16, isOutput=False)
    bq = nc.declare_dram_parameter("bq", [128, NHC], f32, isOutput=False)
    bk = nc.declare_dram_parameter("bk", [128, NHC], f32, isOutput=False)
    bv = nc.declare_dram_parameter("bv", [1, 512], bf16, isOutput=False)
    mask = nc.declare_dram_parameter("mask", [128, 128], bf16, isOutput=False)
    out = nc.declare_dram_parameter("out", [S, E], f32, isOutput=True)
    if taps:
        t_q = nc.declare_dram_parameter("t_q", [128, S], bf16, isOutput=True)
        t_k = nc.declare_dram_parameter("t_k", [128, S], bf16, isOutput=True)
        t_v = nc.declare_dram_parameter("t_v", [128, HPC * VW], bf16,
                                        isOutput=True)
        t_pt = nc.declare_dram_parameter("t_pt", [128, 512], bf16,
                                         isOutput=True)
        t_dn = nc.declare_dram_parameter("t_dn", [VW, 512], bf16,
                                         isOutput=True)
        t_rb = nc.declare_dram_parameter("t_rb", [D, 512], bf16,
                                         isOutput=True)
        t_ao = nc.declare_dram_parameter("t_ao", [128, NHC * S], bf16,
                                         isOutput=True)

    from contextlib import ExitStack
    with TileContext(nc) as tc:
        with ExitStack() as ctx:
            cpool = ctx.enter_context(tc.tile_pool(name="const", bufs=1))
            pt_pool = ctx.enter_context(tc.tile_pool(name="pt", bufs=4))
            praw_pool = ctx.enter_context(tc.tile_pool(name="praw", bufs=2))
            denb_pool = ctx.enter_context(tc.tile_pool(name="denb", bufs=3))
            rbs_pool = ctx.enter_context(tc.tile_pool(name="rbs", bufs=2))
            tmp_pool = ctx.enter_context(tc.tile_pool(name="tmp", bufs=2))
            og_pool = ctx.enter_context(tc.tile_pool(name="og", bufs=3))
            pp_pool = ctx.enter_context(tc.tile_pool(name="pp", bufs=2,
                                                     space="PSUM"))
            st_pool = ctx.enter_context(tc.tile_pool(name="st", bufs=3,
                                                     space="PSUM"))
            ot_pool = ctx.enter_context(tc.tile_pool(name="ot", bufs=3,
                                                     space="PSUM"))

            # ---- persistent SBUF tensors.  DMA order matters: the first
            # projection group needs wq + x s-block 0, so interleave weight
            # and x loads instead of queueing all 4 MiB of x first.
            xb = cpool.tile([128, 8 * S], bf16, name="xb")
            xbv = xb.rearrange("p (c j) -> p c j", c=8)
            wq_t = cpool.tile([128, 8 * 512], bf16, name="wq_t")
            wk_t = cpool.tile([128, 8 * 512], bf16, name="wk_t")
            wv_t = cpool.tile([128, 8 * 512], bf16, name="wv_t")

            def load_x(sb):
                nc.sync.dma_start(
                    out=xbv[:, :, sb * 512:(sb + 1) * 512],
                    in_=xT[:, sb * 512:(sb + 1) * 512]
                        .rearrange("(c p) j -> p c j", p=128))

            def load_w(w_d, w_s):
                nc.sync.dma_start(
                    out=w_s.rearrange("p (c j) -> p c j", c=8),
                    in_=w_d.rearrange("(c p) j -> p c j", p=128))

            load_w(wq, wq_t)
            load_x(0)
            load_w(wk, wk_t)
            load_x(1)
            load_w(wv, wv_t)
            load_x(2)
            load_x(3)
            wo_t = cpool.tile([128, 4 * E], bf16, name="wo_t")
            nc.sync.dma_start(
                out=wo_t.rearrange("p (c j) -> p c j", c=4),
                in_=wo.rearrange("(c p) j -> p c j", p=128))
            bq_t = cpool.tile([128, NHC], f32, name="bq_t")
            bk_t = cpool.tile([128, NHC], f32, name="bk_t")
            nc.sync.dma_start(out=bq_t[:], in_=bq[:, :])
            nc.sync.dma_start(out=bk_t[:], in_=bk[:, :])
            bv_t = cpool.tile([1, 512], bf16, name="bv_t")
            nc.sync.dma_start(out=bv_t[:], in_=bv[:, :])
            mask_t = cpool.tile([128, 128], bf16, name="mask_t")
            nc.sync.dma_start(out=mask_t[:], in_=mask[:, :])
            ones1 = cpool.tile([1, 128], bf16, name="ones1")
            nc.vector.memset(ones1[:], 1.0)
            ones_bf = cpool.tile([128, D], bf16, name="ones_bf")
            nc.vector.memset(ones_bf[:], 1.0)

            qt = cpool.tile([128, NHC * S], bf16, name="qt")
            kt = cpool.tile([128, NHC * S], bf16, name="kt")
            vsb = cpool.tile([128, NSC * HPC * VW], bf16, name="vsb")
            aot = cpool.tile([128, NHC * S], bf16, name="aot")
            # ones columns: vsb[(kc, h, D)] = 1
            nc.gpsimd.memset(
                vsb.rearrange("p (c h x) -> p c h x", c=NSC, h=HPC)
                   [:, :, :, D:D + 1], 1.0)

            # ---- projection emission, sliced into ~2-matmul closures ------
            def qk_closures(w_t, b_t, dst, hc, stile):
                state = {}

                def mm(ec):
                    nc.tensor.matmul(
                        state["ps"][:],
                        w_t[:, ec * 512 + hc * 128: ec * 512 + (hc + 1) * 128],
                        xbv[:, ec, stile * 512:(stile + 1) * 512],
                        start=(ec == 0), stop=(ec == 7))

                def c0():
                    state["ps"] = pp_pool.tile(
                        [128, 512], f32, tag="pp",
                        name=f"qk{hc}_{stile}_{w_t.tensor.name}")
                    mm(0); mm(1)

                def c3():
                    mm(6); mm(7)
                    nc.scalar.activation(
                        dst[:, hc * S + stile * 512:
                            hc * S + (stile + 1) * 512],
                        state["ps"][:], AF.Identity,
                        bias=b_t[:, hc:hc + 1], scale=1.0)

                return [c0, lambda: (mm(2), mm(3)), lambda: (mm(4), mm(5)),
                        c3]

            def v_closures(sc):
                # row-major V for all 8 heads, s-chunk sc
                state = {}

                def mm(ec):
                    nc.tensor.matmul(
                        state["vp"][:],
                        xbv[:, ec, sc * 128:(sc + 1) * 128],
                        wv_t[:, ec * 512:(ec + 1) * 512],
                        start=(ec == 0), stop=False)

                def c0():
                    state["vp"] = pp_pool.tile([128, 512], f32, tag="pp",
                                               name=f"vp{sc}")
                    mm(0); mm(1)

                def c3():
                    mm(6); mm(7)
                    nc.tensor.matmul(state["vp"][:], ones1[:, :], bv_t[:, :],
                                     start=False, stop=True)
                    dst = vsb[:, sc * HPC * VW: (sc + 1) * HPC * VW] \
                        .rearrange("p (h x) -> p h x", h=HPC)[:, :, 0:D]
                    nc.vector.tensor_copy(
                        dst, state["vp"].rearrange("p (h x) -> p h x",
                                                   h=HPC))

                return [c0, lambda: (mm(2), mm(3)), lambda: (mm(4), mm(5)),
                        c3]

            def proj_qk_closures(hc):
                gs = []
                for (w_t, b_t, dst) in ((wq_t, bq_t, qt), (wk_t, bk_t, kt)):
                    for stile in range(4):
                        gs.extend(qk_closures(w_t, b_t, dst, hc, stile))
                return gs

            # ---- attention unit -------------------------------------------
            def attention_unit(h, jq, slot=None):
                hc, off = h >> 1, (h & 1) * D
                nkt = 4 * (jq + 1)
                ot = ot_pool.tile([VW, 512], f32, tag="ot", name=f"ot{h}_{jq}")
                for ki in range(nkt):
                    r = ki - 4 * jq
                    wdt = 512 - 128 * r if r >= 0 else 512
                    st = st_pool.tile([128, 512], f32, tag="st",
                                      name=f"st{h}_{jq}_{ki}")
                    pt = pt_pool.tile([128, 512], bf16, tag="pt",
                                      name=f"pt{h}_{jq}_{ki}")
                    nc.tensor.matmul(
                        st[:, 0:wdt],
                        kt[off:off + D,
                           hc * S + ki * 128: hc * S + (ki + 1) * 128],
                        qt[off:off + D,
                           hc * S + jq * 512 + (512 - wdt):
                           hc * S + (jq + 1) * 512],
                        start=True, stop=True)
                    if r >= 0:
                        # single exp over the visible range, then mask the
                        # boundary 128 cols with an in-place DVE multiply
                        nc.scalar.activation(pt[:, 0:wdt], st[:, 0:wdt],
                                             AF.Exp, scale=0.125)
                        nc.vector.tensor_mul(pt[:, 0:128], pt[:, 0:128],
                                             mask_t[:])
                    else:
                        nc.scalar.activation(pt[:], st[:],
                                             AF.Exp, scale=0.125)
                    if taps and h == 0 and jq == 0 and ki == 0:
                        nc.sync.dma_start(out=t_pt[:, :], in_=pt[:, :])
                    nc.tensor.matmul(
                        ot[:, (512 - wdt):512],
                        vsb[:, ki * HPC * VW + h * VW:
                            ki * HPC * VW + (h + 1) * VW],
                        pt[:, 0:wdt],
                        start=(ki == 0), stop=(ki == nkt - 1))
                    if slot is not None:
                        slot()
                # normalize: row D of ot = sum_k exp.  Reciprocal on the DVE
                # (fast approx), broadcast across partitions via a contract-1
                # PE matmul with a ones column (baseline-proven pattern).
                # reciprocal_approx_fast (custom DVE uop) emits garbage on
                # HW via this compile path — use the bit-exact InstReciprocal
                # (baseline-proven reading PSUM), writing bf16 directly.
                # The 3.3us InstReciprocal would stall the in-order PE queue
                # at the rb matmul; the caller defers the returned finisher
                # until after the next unit's chunks are emitted.
                den_bf = denb_pool.tile([VW, 512], bf16, tag="denb",
                                        name=f"denb{h}_{jq}")
                with nc.allow_low_precision(reason="bf16 recip feeds PE"):
                    nc.vector.reciprocal(den_bf[D:D + 1, :], ot[D:D + 1, :])

                def finish(h=h, jq=jq, hc=hc, off=off, ot=ot, den_bf=den_bf):
                    rbt = pp_pool.tile([128, 512], f32, tag="pp",
                                       name=f"rb{h}_{jq}")
                    nc.tensor.matmul(rbt[0:D, :], ones_bf[D:D + 1, 0:D],
                                     den_bf[D:D + 1, :], start=True,
                                     stop=True)
                    rbs = rbs_pool.tile([D, 512], bf16, tag="rbs",
                                        name=f"rbs{h}_{jq}")
                    nc.vector.tensor_copy(rbs[:], rbt[0:D, :])
                    if taps and h == 0 and jq == 0:
                        nc.sync.dma_start(out=t_dn[:, :], in_=den_bf[:, :])
                        nc.sync.dma_start(out=t_rb[:, :], in_=rbs[:, :])
                    cols = slice(hc * S + jq * 512, hc * S + (jq + 1) * 512)
                    if off == 0:
                        nc.vector.tensor_mul(aot[0:D, cols], ot[0:D, :],
                                             rbs[:])
                    else:
                        # engines cannot shift partition base; route odd-head
                        # rows to partitions 64..127 via SBUF->SBUF DMA.
                        tmp = tmp_pool.tile([D, 512], bf16, tag="tmp",
                                            name=f"tmp{h}_{jq}")
                        nc.vector.tensor_mul(tmp[:], ot[0:D, :], rbs[:])
                        nc.sync.dma_start(out=aot[D:2 * D, cols], in_=tmp[:])

                return finish

            # ---- schedule -------------------------------------------------
            # pre-phase: QK(0) + V chunks 0..9 emitted inline; the rest are
            # sliced into ~2-matmul closures injected after every attention
            # chunk to keep the PE streaming (max pstate) while the Scalar
            # engine runs the exps.
            for g in proj_qk_closures(0):
                g()
            for sc in range(10):
                for g in v_closures(sc):
                    g()
            fillers = []
            for sc in range(10, 16):
                fillers.extend(v_closures(sc))
            for hc in range(1, 4):
                fillers.extend(proj_qk_closures(hc))

            def slot(n=1):
                for _ in range(n):
                    if fillers:
                        fillers.pop(0)()

            # ---- output projection (one 512-col q-slice at a time) -------
            def outproj_jq(jq):
                for t in range(4 * jq, 4 * jq + 4):
                    for eh in range(2):
                        op = pp_pool.tile([128, 512], f32, tag="pp",
                                          name=f"op{t}_{eh}")
                        for hcc in range(4):
                            nc.tensor.matmul(
                                op[:],
                                aot[:, hcc * S + t * 128:
                                    hcc * S + (t + 1) * 128],
                                wo_t[:, hcc * E + eh * 512:
                                     hcc * E + (eh + 1) * 512],
                                start=(hcc == 0), stop=(hcc == 3))
                        og = og_pool.tile([128, 512], f32, tag="og",
                                          name=f"og{t}_{eh}")
                        nc.scalar.copy(og[:], op[:])
                        nc.sync.dma_start(
                            out=out[t * 128:(t + 1) * 128,
                                    eh * 512:(eh + 1) * 512],
                            in_=og[:])

            # hc 0-2 run head-major; the last head pair runs jq-major so
            # each out-projection q-slice can be emitted as soon as all 8
            # heads' normalized rows for that slice exist, hiding the
            # out-projection tail under the last attention units.
            units = [(2 * hc + hh, jq) for hc in range(3) for hh in range(2)
                     for jq in range(4)]
            units += [(6 + hh, jq) for jq in range(4) for hh in range(2)]

            done = set()
            emitted = set()

            def after_pop(tag):
                done.add(tag)
                for jq in range(4):
                    if jq not in emitted and (6, jq) in done \
                            and (7, jq) in done:
                        emitted.add(jq)
                        outproj_jq(jq)

            from collections import deque
            pending = deque()
            for (h, jq) in units:
                fin = attention_unit(h, jq)
                pending.append(((h, jq), fin))
                slot(8)
                # defer each unit's norm-finish by 2 units so the 3.3us
                # reciprocal never blocks the in-order PE queue
                if len(pending) > 2:
                    tag, f = pending.popleft()
                    f()
                    after_pop(tag)
            while pending:
                tag, f = pending.popleft()
                f()
                after_pop(tag)
            while fillers:
                fillers.pop(0)()
            assert emitted == {0, 1, 2, 3}

            if taps:
                nc.sync.dma_start(out=t_q[:, :], in_=qt[:, 0:S])
                nc.sync.dma_start(out=t_k[:, :], in_=kt[:, 0:S])
                nc.sync.dma_start(out=t_v[:, :], in_=vsb[:, 0:HPC * VW])
                nc.sync.dma_start(out=t_ao[:, :], in_=aot[:, :])
    nc.compile()
    return nc


def _run(inputs, trace=False, trace_kwargs=None):
    x = np.asarray(inputs["x"], dtype=np.float32)
    wq_w = np.asarray(inputs["wq_w"], dtype=np.float32)
    wq_b = np.asarray(inputs["wq_b"], dtype=np.float32)
    wk_w = np.asarray(inputs["wk_w"], dtype=np.float32)
    wk_b = np.asarray(inputs["wk_b"], dtype=np.float32)
    wv_w = np.asarray(inputs["wv_w"], dtype=np.float32)
    wv_b = np.asarray(inputs["wv_b"], dtype=np.float32)
    wo_w = np.asarray(inputs["wo_w"], dtype=np.float32)
    wo_b = np.asarray(inputs["wo_b"], dtype=np.float32)

    maskh = np.triu(np.ones((128, 128), dtype=np.float32)).astype(BF)

    if "nc" not in _CACHE:
        _CACHE["nc"] = build_nc()
    nc = _CACHE["nc"]

    in_maps = []
    for c in range(NCORES):
        b, hg = c // 2, c % 2
        sl = slice(hg * 512, hg * 512 + 512)
        in_maps.append({
            "xT": np.ascontiguousarray(x[b].T).astype(BF),
            "wq": np.ascontiguousarray(wq_w[:, sl]).astype(BF),
            "wk": np.ascontiguousarray(wk_w[:, sl]).astype(BF),
            "wv": np.ascontiguousarray(wv_w[:, sl]).astype(BF),
            "wo": np.ascontiguousarray(wo_w[sl, :]).astype(BF),
            "bq": np.ascontiguousarray(wq_b[sl].reshape(NHC, 128).T),
            "bk": np.ascontiguousarray(wk_b[sl].reshape(NHC, 128).T),
            "bv": wv_b[sl].reshape(1, 512).astype(BF),
            "mask": maskh,
        })

    kwargs = {}
    if trace:
        kwargs["trace"] = True
        if trace_kwargs:
            kwargs.update(trace_kwargs)
    res = run_bass_kernel_spmd(nc, in_maps, list(range(NCORES)), **kwargs)

    full = np.empty((B, S, E), dtype=np.float32)
    for b in range(B):
        full[b] = (res.results[2 * b]["out"].astype(np.float32)
                   + res.results[2 * b + 1]["out"].astype(np.float32)
                   + wo_b[None, :])
    return full, res


def kernel(**inputs):
    out, _ = _run(inputs, trace=False)
    return out


if __name__ == "__main__":
    rng = np.random.default_rng(0)
    ins = {
        "x": rng.standard_normal((B, S, E), dtype=np.float32),
        "wq_w": rng.standard_normal((E, E), dtype=np.float32) / 32,
        "wq_b": np.zeros(E, np.float32),
        "wk_w": rng.standard_normal((E, E), dtype=np.float32) / 32,
        "wk_b": np.zeros(E, np.float32),
        "wv_w": rng.standard_normal((E, E), dtype=np.float32) / 32,
        "wv_b": np.zeros(E, np.float32),
        "wo_w": rng.standard_normal((E, E), dtype=np.float32) / 32,
        "wo_b": np.zeros(E, np.float32),
    }
    out = kernel(**ins)
    print("ok", out.shape, out.dtype)


# revision 48
# speedup vs baseline: 1.0331x; 1.0331x over previous
"""Causal multi-head self-attention on 8 Trainium2 NeuronCores.

Sharding: (batch, head-group).  Core c owns batch c//2 and heads
(c%2)*8 .. +8 (= 512 of the 1024 feature dims).  Each core:
  - projects Q^T/K^T (dim-major) and V (row-major) for its 512 dims,
  - runs causal attention for its 8 heads over its batch,
  - computes the partial output projection  attn_out @ wo_w[dims, :].
The host sums the 2 partial projections per batch and adds wo_b.

All matmuls run in bf16 (fp32 PSUM accumulation); fp32r was measured
executing as multi-pass fp32_mode=HIGH with heavy power throttling.
Scores are computed transposed (S^T[k, q]); an interleaved ones-column
per head in the V tile makes the P@V matmul emit softmax denominators.
Normalization uses reciprocal_approx_fast + gpsimd partition_broadcast
(no PE broadcast matmuls, no slow InstReciprocal).
Projection matmul groups are interleaved between attention units so the
PE stays busy while the Scalar engine streams the exps.
"""
import sys
sys.path.insert(0, "/opt/trn_rl_repo")

import numpy as np
import ml_dtypes
import concourse.bass as bass
import concourse.mybir as mybir
from concourse import bacc
from concourse.tile import TileContext
from concourse.bass_utils import run_bass_kernel_spmd

B, S, E = 4, 2048, 1024
H, D = 16, 64
NCORES = 8
HPC = 8                 # heads per core
NHC = 4                 # 128-dim chunks per core (2 heads each)
VW = D + 1              # v cols per head incl ones column
NSC = S // 128          # 16 k/s chunks of 128

f32 = mybir.dt.float32
bf16 = mybir.dt.bfloat16
AF = mybir.ActivationFunctionType
BF = ml_dtypes.bfloat16

_CACHE = {}


def build_nc(taps=False):
    nc = bacc.Bacc("TRN2", target_bir_lowering=False, debug=False)

    xT = nc.declare_dram_parameter("xT", [E, S], bf16, isOutput=False)
    wq = nc.declare_dram_parameter("wq", [E, 512], bf16, isOutput=False)
    wk = nc.declare_dram_parameter("wk", [E, 512], bf16, isOutput=False)
    wv = nc.declare_dram_parameter("wv", [E, 512], bf16, isOutput=False)
    wo = nc.declare_dram_parameter("wo", [512, E], bf16, isOutput=False)
    bq = nc.declare_dram_parameter("bq", [128, NHC], f32, isOutput=False)
    bk = nc.declare_dram_parameter("bk", [128, NHC], f32, isOutput=False)
    bv = nc.declare_dram_parameter("bv", [1, 512], bf16, isOutput=False)
    mask = nc.declare_dram_parameter("mask", [128, 128], bf16, isOutput=False)
    out = nc.declare_dram_parameter("out", [S, E], f32, isOutput=True)
    if taps:
        t_q = nc.declare_dram_parameter("t_q", [128, S], bf16, isOutput=True)
        t_k = nc.declare_dram_parameter("t_k", [128, S], bf16, isOutput=True)
        t_v = nc.declare_dram_parameter("t_v", [128, HPC * VW], bf16,
                                        isOutput=True)
        t_pt = nc.declare_dram_parameter("t_pt", [128, 512], bf16,
                                         isOutput=True)
        t_dn = nc.declare_dram_parameter("t_dn", [VW, 512], bf16,
                                         isOutput=True)
        t_rb = nc.declare_dram_parameter("t_rb", [D, 512], bf16,
                                         isOutput=True)
        t_ao = nc.declare_dram_parameter("t_ao", [128, NHC * S], bf16,
                                         isOutput=True)

    from contextlib import ExitStack
    with TileContext(nc) as tc:
        with ExitStack() as ctx:
            cpool = ctx.enter_context(tc.tile_pool(name="const", bufs=1))
            pt_pool = ctx.enter_context(tc.tile_pool(name="pt", bufs=4))
            praw_pool = ctx.enter_context(tc.tile_pool(name="praw", bufs=2))
            denb_pool = ctx.enter_context(tc.tile_pool(name="denb", bufs=3))
            rbs_pool = ctx.enter_context(tc.tile_pool(name="rbs", bufs=2))
            tmp_pool = ctx.enter_context(tc.tile_pool(name="tmp", bufs=2))
            og_pool = ctx.enter_context(tc.tile_pool(name="og", bufs=3))
            pp_pool = ctx.enter_context(tc.tile_pool(name="pp", bufs=2,
                                                     space="PSUM"))
            st_pool = ctx.enter_context(tc.tile_pool(name="st", bufs=3,
                                                     space="PSUM"))
            ot_pool = ctx.enter_context(tc.tile_pool(name="ot", bufs=3,
                                                     space="PSUM"))

            # ---- persistent SBUF tensors.  DMA order matters: the first
            # projection group needs wq + x s-block 0, so interleave weight
            # and x loads instead of queueing all 4 MiB of x first.
            xb = cpool.tile([128, 8 * S], bf16, name="xb")
            xbv = xb.rearrange("p (c j) -> p c j", c=8)
            wq_t = cpool.tile([128, 8 * 512], bf16, name="wq_t")
            wk_t = cpool.tile([128, 8 * 512], bf16, name="wk_t")
            wv_t = cpool.tile([128, 8 * 512], bf16, name="wv_t")

            def load_x(sb):
                nc.sync.dma_start(
                    out=xbv[:, :, sb * 512:(sb + 1) * 512],
                    in_=xT[:, sb * 512:(sb + 1) * 512]
                        .rearrange("(c p) j -> p c j", p=128))

            def load_w(w_d, w_s):
                nc.sync.dma_start(
                    out=w_s.rearrange("p (c j) -> p c j", c=8),
                    in_=w_d.rearrange("(c p) j -> p c j", p=128))

            load_w(wq, wq_t)
            load_x(0)
            load_w(wk, wk_t)
            load_x(1)
            load_w(wv, wv_t)
            load_x(2)
            load_x(3)
            wo_t = cpool.tile([128, 4 * E], bf16, name="wo_t")
            nc.sync.dma_start(
                out=wo_t.rearrange("p (c j) -> p c j", c=4),
                in_=wo.rearrange("(c p) j -> p c j", p=128))
            bq_t = cpool.tile([128, NHC], f32, name="bq_t")
            bk_t = cpool.tile([128, NHC], f32, name="bk_t")
            nc.sync.dma_start(out=bq_t[:], in_=bq[:, :])
            nc.sync.dma_start(out=bk_t[:], in_=bk[:, :])
            bv_t = cpool.tile([1, 512], bf16, name="bv_t")
            nc.sync.dma_start(out=bv_t[:], in_=bv[:, :])
            mask_t = cpool.tile([128, 128], bf16, name="mask_t")
            nc.sync.dma_start(out=mask_t[:], in_=mask[:, :])
            ones1 = cpool.tile([1, 128], bf16, name="ones1")
            nc.vector.memset(ones1[:], 1.0)
            ones_bf = cpool.tile([128, D], bf16, name="ones_bf")
            nc.vector.memset(ones_bf[:], 1.0)

            qt = cpool.tile([128, NHC * S], bf16, name="qt")
            kt = cpool.tile([128, NHC * S], bf16, name="kt")
            vsb = cpool.tile([128, NSC * HPC * VW], bf16, name="vsb")
            aot = cpool.tile([128, NHC * S], bf16, name="aot")
            # ones columns: vsb[(kc, h, D)] = 1
            nc.gpsimd.memset(
                vsb.rearrange("p (c h x) -> p c h x", c=NSC, h=HPC)
                   [:, :, :, D:D + 1], 1.0)

            # ---- projection emission, sliced into ~2-matmul closures ------
            def qk_closures(w_t, b_t, dst, hc, stile):
                state = {}

                def mm(ec):
                    nc.tensor.matmul(
                        state["ps"][:],
                        w_t[:, ec * 512 + hc * 128: ec * 512 + (hc + 1) * 128],
                        xbv[:, ec, stile * 512:(stile + 1) * 512],
                        start=(ec == 0), stop=(ec == 7))

                def c0():
                    state["ps"] = pp_pool.tile(
                        [128, 512], f32, tag="pp",
                        name=f"qk{hc}_{stile}_{w_t.tensor.name}")
                    mm(0); mm(1)

                def c3():
                    mm(6); mm(7)
                    nc.scalar.activation(
                        dst[:, hc * S + stile * 512:
                            hc * S + (stile + 1) * 512],
                        state["ps"][:], AF.Identity,
                        bias=b_t[:, hc:hc + 1], scale=1.0)

                return [c0, lambda: (mm(2), mm(3)), lambda: (mm(4), mm(5)),
                        c3]

            def v_closures(sc):
                # row-major V for all 8 heads, s-chunk sc
                state = {}

                def mm(ec):
                    nc.tensor.matmul(
                        state["vp"][:],
                        xbv[:, ec, sc * 128:(sc + 1) * 128],
                        wv_t[:, ec * 512:(ec + 1) * 512],
                        start=(ec == 0), stop=False)

                def c0():
                    state["vp"] = pp_pool.tile([128, 512], f32, tag="pp",
                                               name=f"vp{sc}")
                    mm(0); mm(1)

                def c3():
                    mm(6); mm(7)
                    nc.tensor.matmul(state["vp"][:], ones1[:, :], bv_t[:, :],
                                     start=False, stop=True)
                    dst = vsb[:, sc * HPC * VW: (sc + 1) * HPC * VW] \
                        .rearrange("p (h x) -> p h x", h=HPC)[:, :, 0:D]
                    nc.vector.tensor_copy(
                        dst, state["vp"].rearrange("p (h x) -> p h x",
                                                   h=HPC))

                return [c0, lambda: (mm(2), mm(3)), lambda: (mm(4), mm(5)),
                        c3]

            def proj_qk_closures(hc):
                gs = []
                for (w_t, b_t, dst) in ((wq_t, bq_t, qt), (wk_t, bk_t, kt)):
                    for stile in range(4):
                        gs.extend(qk_closures(w_t, b_t, dst, hc, stile))
                return gs

            # ---- attention unit -------------------------------------------
            def attention_unit(h, jq, slot=None):
                hc, off = h >> 1, (h & 1) * D
                nkt = 4 * (jq + 1)
                ot = ot_pool.tile([VW, 512], f32, tag="ot", name=f"ot{h}_{jq}")
                for ki in range(nkt):
                    r = ki - 4 * jq
                    wdt = 512 - 128 * r if r >= 0 else 512
                    st = st_pool.tile([128, 512], f32, tag="st",
                                      name=f"st{h}_{jq}_{ki}")
                    pt = pt_pool.tile([128, 512], bf16, tag="pt",
                                      name=f"pt{h}_{jq}_{ki}")
                    nc.tensor.matmul(
                        st[:, 0:wdt],
                        kt[off:off + D,
                           hc * S + ki * 128: hc * S + (ki + 1) * 128],
                        qt[off:off + D,
                           hc * S + jq * 512 + (512 - wdt):
                           hc * S + (jq + 1) * 512],
                        start=True, stop=True)
                    if r >= 0:
                        # boundary 128 cols: exp to staging, then masked mul
                        if wdt > 128:
                            nc.scalar.activation(pt[:, 128:wdt],
                                                 st[:, 128:wdt],
                                                 AF.Exp, scale=0.125)
                        praw = praw_pool.tile([128, 128], bf16, tag="praw",
                                              name=f"praw{h}_{jq}_{ki}")
                        nc.scalar.activation(praw[:], st[:, 0:128],
                                             AF.Exp, scale=0.125)
                        nc.vector.tensor_mul(pt[:, 0:128], praw[:],
                                             mask_t[:])
                    else:
                        nc.scalar.activation(pt[:], st[:],
                                             AF.Exp, scale=0.125)
                    if taps and h == 0 and jq == 0 and ki == 0:
                        nc.sync.dma_start(out=t_pt[:, :], in_=pt[:, :])
                    nc.tensor.matmul(
                        ot[:, (512 - wdt):512],
                        vsb[:, ki * HPC * VW + h * VW:
                            ki * HPC * VW + (h + 1) * VW],
                        pt[:, 0:wdt],
                        start=(ki == 0), stop=(ki == nkt - 1))
                    if slot is not None:
                        slot()
                # normalize: row D of ot = sum_k exp.  Reciprocal on the DVE
                # (fast approx), broadcast across partitions via a contract-1
                # PE matmul with a ones column (baseline-proven pattern).
                # reciprocal_approx_fast (custom DVE uop) emits garbage on
                # HW via this compile path — use the bit-exact InstReciprocal
                # (baseline-proven reading PSUM), writing bf16 directly.
                # The 3.3us InstReciprocal would stall the in-order PE queue
                # at the rb matmul; the caller defers the returned finisher
                # until after the next unit's chunks are emitted.
                den_bf = denb_pool.tile([VW, 512], bf16, tag="denb",
                                        name=f"denb{h}_{jq}")
                with nc.allow_low_precision(reason="bf16 recip feeds PE"):
                    nc.vector.reciprocal(den_bf[D:D + 1, :], ot[D:D + 1, :])

                def finish(h=h, jq=jq, hc=hc, off=off, ot=ot, den_bf=den_bf):
                    rbt = pp_pool.tile([128, 512], f32, tag="pp",
                                       name=f"rb{h}_{jq}")
                    nc.tensor.matmul(rbt[0:D, :], ones_bf[D:D + 1, 0:D],
                                     den_bf[D:D + 1, :], start=True,
                                     stop=True)
                    rbs = rbs_pool.tile([D, 512], bf16, tag="rbs",
                                        name=f"rbs{h}_{jq}")
                    nc.vector.tensor_copy(rbs[:], rbt[0:D, :])
                    if taps and h == 0 and jq == 0:
                        nc.sync.dma_start(out=t_dn[:, :], in_=den_bf[:, :])
                        nc.sync.dma_start(out=t_rb[:, :], in_=rbs[:, :])
                    cols = slice(hc * S + jq * 512, hc * S + (jq + 1) * 512)
                    if off == 0:
                        nc.vector.tensor_mul(aot[0:D, cols], ot[0:D, :],
                                             rbs[:])
                    else:
                        # engines cannot shift partition base; route odd-head
                        # rows to partitions 64..127 via SBUF->SBUF DMA.
                        tmp = tmp_pool.tile([D, 512], bf16, tag="tmp",
                                            name=f"tmp{h}_{jq}")
                        nc.vector.tensor_mul(tmp[:], ot[0:D, :], rbs[:])
                        nc.sync.dma_start(out=aot[D:2 * D, cols], in_=tmp[:])

                return finish

            # ---- schedule -------------------------------------------------
            # pre-phase: QK(0) + V chunks 0..9 emitted inline; the rest are
            # sliced into ~2-matmul closures injected after every attention
            # chunk to keep the PE streaming (max pstate) while the Scalar
            # engine runs the exps.
            for g in proj_qk_closures(0):
                g()
            for sc in range(10):
                for g in v_closures(sc):
                    g()
            fillers = []
            for sc in range(10, 16):
                fillers.extend(v_closures(sc))
            for hc in range(1, 4):
                fillers.extend(proj_qk_closures(hc))

            def slot(n=1):
                for _ in range(n):
                    if fillers:
                        fillers.pop(0)()

            # ---- output projection (one 512-col q-slice at a time) -------
            def outproj_jq(jq):
                for t in range(4 * jq, 4 * jq + 4):
                    for eh in range(2):
                        op = pp_pool.tile([128, 512], f32, tag="pp",
                                          name=f"op{t}_{eh}")
                        for hcc in range(4):
                            nc.tensor.matmul(
                                op[:],
                                aot[:, hcc * S + t * 128:
                                    hcc * S + (t + 1) * 128],
                                wo_t[:, hcc * E + eh * 512:
                                     hcc * E + (eh + 1) * 512],
                                start=(hcc == 0), stop=(hcc == 3))
                        og = og_pool.tile([128, 512], f32, tag="og",
                                          name=f"og{t}_{eh}")
                        nc.scalar.copy(og[:], op[:])
                        nc.sync.dma_start(
                            out=out[t * 128:(t + 1) * 128,
                                    eh * 512:(eh + 1) * 512],
                            in_=og[:])

            # hc 0-2 run head-major; the last head pair runs jq-major so
            # each out-projection q-slice can be emitted as soon as all 8
            # heads' normalized rows for that slice exist, hiding the
            # out-projection tail under the last attention units.
            units = [(2 * hc + hh, jq) for hc in range(3) for hh in range(2)
                     for jq in range(4)]
            units += [(6 + hh, jq) for jq in range(4) for hh in range(2)]

            done = set()
            emitted = set()

            def after_pop(tag):
                done.add(tag)
                for jq in range(4):
                    if jq not in emitted and (6, jq) in done \
                            and (7, jq) in done:
                        emitted.add(jq)
                        outproj_jq(jq)

            from collections import deque
            pending = deque()
            for (h, jq) in units:
                fin = attention_unit(h, jq)
                pending.append(((h, jq), fin))
                slot(8)
                # defer each unit's norm-finish by 2 units so the 3.3us
                # reciprocal never blocks the in-order PE queue
                if len(pending) > 2:
                    tag, f = pending.popleft()
                    f()
                    after_pop(tag)
            while pending:
                tag, f = pending.popleft()
                f()
                after_pop(tag)
            while fillers:
                fillers.pop(0)()
            assert emitted == {0, 1, 2, 3}

            if taps:
                nc.sync.dma_start(out=t_q[:, :], in_=qt[:, 0:S])
                nc.sync.dma_start(out=t_k[:, :], in_=kt[:, 0:S])
                nc.sync.dma_start(out=t_v[:, :], in_=vsb[:, 0:HPC * VW])
                nc.sync.dma_start(out=t_ao[:, :], in_=aot[:, :])
    nc.compile()
    return nc


def _run(inputs, trace=False, trace_kwargs=None):
    x = np.asarray(inputs["x"], dtype=np.float32)
    wq_w = np.asarray(inputs["wq_w"], dtype=np.float32)
    wq_b = np.asarray(inputs["wq_b"], dtype=np.float32)
    wk_w = np.asarray(inputs["wk_w"], dtype=np.float32)
    wk_b = np.asarray(inputs["wk_b"], dtype=np.float32)
    wv_w = np.asarray(inputs["wv_w"], dtype=np.float32)
    wv_b = np.asarray(inputs["wv_b"], dtype=np.float32)
    wo_w = np.asarray(inputs["wo_w"], dtype=np.float32)
    wo_b = np.asarray(inputs["wo_b"], dtype=np.float32)

    maskh = np.triu(np.ones((128, 128), dtype=np.float32)).astype(BF)

    if "nc" not in _CACHE:
        _CACHE["nc"] = build_nc()
    nc = _CACHE["nc"]

    in_maps = []
    for c in range(NCORES):
        b, hg = c // 2, c % 2
        sl = slice(hg * 512, hg * 512 + 512)
        in_maps.append({
            "xT": np.ascontiguousarray(x[b].T).astype(BF),
            "wq": np.ascontiguousarray(wq_w[:, sl]).astype(BF),
            "wk": np.ascontiguousarray(wk_w[:, sl]).astype(BF),
            "wv": np.ascontiguousarray(wv_w[:, sl]).astype(BF),
            "wo": np.ascontiguousarray(wo_w[sl, :]).astype(BF),
            "bq": np.ascontiguousarray(wq_b[sl].reshape(NHC, 128).T),
            "bk": np.ascontiguousarray(wk_b[sl].reshape(NHC, 128).T),
            "bv": wv_b[sl].reshape(1, 512).astype(BF),
            "mask": maskh,
        })

    kwargs = {}
    if trace:
        kwargs["trace"] = True
        if trace_kwargs:
            kwargs.update(trace_kwargs)
    res = run_bass_kernel_spmd(nc, in_maps, list(range(NCORES)), **kwargs)

    full = np.empty((B, S, E), dtype=np.float32)
    for b in range(B):
        full[b] = (res.results[2 * b]["out"].astype(np.float32)
                   + res.results[2 * b + 1]["out"].astype(np.float32)
                   + wo_b[None, :])
    return full, res


def kernel(**inputs):
    out, _ = _run(inputs, trace=False)
    return out


if __name__ == "__main__":
    rng = np.random.default_rng(0)
    ins = {
        "x": rng.standard_normal((B, S, E), dtype=np.float32),
        "wq_w": rng.standard_normal((E, E), dtype=np.float32) / 32,
        "wq_b": np.zeros(E, np.float32),
        "wk_w": rng.standard_normal((E, E), dtype=np.float32) / 32,
        "wk_b": np.zeros(E, np.float32),
        "wv_w": rng.standard_normal((E, E), dtype=np.float32) / 32,
        "wv_b": np.zeros(E, np.float32),
        "wo_w": rng.standard_normal((E, E), dtype=np.float32) / 32,
        "wo_b": np.zeros(E, np.float32),
    }
    out = kernel(**ins)
    print("ok", out.shape, out.dtype)


# revision 52
# speedup vs baseline: 1.0634x; 1.0293x over previous
"""Causal multi-head self-attention on 8 Trainium2 NeuronCores.

Sharding: (batch, head-group).  Core c owns batch c//2 and heads
(c%2)*8 .. +8 (= 512 of the 1024 feature dims).  Each core:
  - projects Q^T/K^T (dim-major) and V (row-major) for its 512 dims,
  - runs causal attention for its 8 heads over its batch,
  - computes the partial output projection  attn_out @ wo_w[dims, :].
The host sums the 2 partial projections per batch and adds wo_b.

All matmuls run in bf16 (fp32 PSUM accumulation); fp32r was measured
executing as multi-pass fp32_mode=HIGH with heavy power throttling.
Scores are computed transposed (S^T[k, q]); an interleaved ones-column
per head in the V tile makes the P@V matmul emit softmax denominators.
Normalization: DVE InstReciprocal -> ones-column PE matmul broadcast ->
per-head multiply, with each unit's norm-finish deferred by two
attention units so the 3.3us reciprocal never blocks the in-order PE
queue.  Projection matmuls are sliced into 2-matmul closures drained
between attention units; the last head pair runs jq-major so each
512-col out-projection slice is emitted as soon as its rows complete,
hiding the output-projection tail under the final attention units.
"""
import sys
sys.path.insert(0, "/opt/trn_rl_repo")

import numpy as np
import ml_dtypes
import concourse.bass as bass
import concourse.mybir as mybir
from concourse import bacc
from concourse.tile import TileContext
from concourse.bass_utils import run_bass_kernel_spmd

B, S, E = 4, 2048, 1024
H, D = 16, 64
NCORES = 8
HPC = 8                 # heads per core
NHC = 4                 # 128-dim chunks per core (2 heads each)
VW = D + 1              # v cols per head incl ones column
NSC = S // 128          # 16 k/s chunks of 128

f32 = mybir.dt.float32
bf16 = mybir.dt.bfloat16
AF = mybir.ActivationFunctionType
BF = ml_dtypes.bfloat16

_CACHE = {}


def build_nc(taps=False):
    nc = bacc.Bacc("TRN2", target_bir_lowering=False, debug=False)

    xT = nc.declare_dram_parameter("xT", [E, S], bf16, isOutput=False)
    wq = nc.declare_dram_parameter("wq", [E, 512], bf16, isOutput=False)
    wk = nc.declare_dram_parameter("wk", [E, 512], bf16, isOutput=False)
    wv = nc.declare_dram_parameter("wv", [E, 512], bf16, isOutput=False)
    wo = nc.declare_dram_parameter("wo", [512, E], bf16, isOutput=False)
    bq = nc.declare_dram_parameter("bq", [128, NHC], f32, isOutput=False)
    bk = nc.declare_dram_parameter("bk", [128, NHC], f32, isOutput=False)
    bv = nc.declare_dram_parameter("bv", [1, 512], bf16, isOutput=False)
    mask = nc.declare_dram_parameter("mask", [128, 128], bf16, isOutput=False)
    out = nc.declare_dram_parameter("out", [S, E], bf16, isOutput=True)
    if taps:
        t_q = nc.declare_dram_parameter("t_q", [128, S], bf16, isOutput=True)
        t_k = nc.declare_dram_parameter("t_k", [128, S], bf16, isOutput=True)
        t_v = nc.declare_dram_parameter("t_v", [128, HPC * VW], bf16,
                                        isOutput=True)
        t_pt = nc.declare_dram_parameter("t_pt", [128, 512], bf16,
                                         isOutput=True)
        t_dn = nc.declare_dram_parameter("t_dn", [VW, 512], bf16,
                                         isOutput=True)
        t_rb = nc.declare_dram_parameter("t_rb", [D, 512], bf16,
                                         isOutput=True)
        t_ao = nc.declare_dram_parameter("t_ao", [128, NHC * S], bf16,
                                         isOutput=True)

    from contextlib import ExitStack
    with TileContext(nc) as tc:
        with ExitStack() as ctx:
            cpool = ctx.enter_context(tc.tile_pool(name="const", bufs=1))
            pt_pool = ctx.enter_context(tc.tile_pool(name="pt", bufs=4))
            praw_pool = ctx.enter_context(tc.tile_pool(name="praw", bufs=2))
            denb_pool = ctx.enter_context(tc.tile_pool(name="denb", bufs=3))
            rbs_pool = ctx.enter_context(tc.tile_pool(name="rbs", bufs=2))
            tmp_pool = ctx.enter_context(tc.tile_pool(name="tmp", bufs=2))
            og_pool = ctx.enter_context(tc.tile_pool(name="og", bufs=3))
            pp_pool = ctx.enter_context(tc.tile_pool(name="pp", bufs=2,
                                                     space="PSUM"))
            st_pool = ctx.enter_context(tc.tile_pool(name="st", bufs=3,
                                                     space="PSUM"))
            ot_pool = ctx.enter_context(tc.tile_pool(name="ot", bufs=3,
                                                     space="PSUM"))

            # ---- persistent SBUF tensors.  DMA order matters: the first
            # projection group needs wq + x s-block 0, so interleave weight
            # and x loads instead of queueing all 4 MiB of x first.
            xb = cpool.tile([128, 8 * S], bf16, name="xb")
            xbv = xb.rearrange("p (c j) -> p c j", c=8)
            wq_t = cpool.tile([128, 8 * 512], bf16, name="wq_t")
            wk_t = cpool.tile([128, 8 * 512], bf16, name="wk_t")
            wv_t = cpool.tile([128, 8 * 512], bf16, name="wv_t")

            def load_x(sb):
                nc.sync.dma_start(
                    out=xbv[:, :, sb * 512:(sb + 1) * 512],
                    in_=xT[:, sb * 512:(sb + 1) * 512]
                        .rearrange("(c p) j -> p c j", p=128))

            def load_w(w_d, w_s, hc):
                # per-head-chunk weight slice (256KB) so the first
                # projection group isn't gated on a full 1MiB weight DMA
                nc.sync.dma_start(
                    out=w_s.rearrange("p (c j) -> p c j", c=8)
                        [:, :, hc * 128:(hc + 1) * 128],
                    in_=w_d[:, hc * 128:(hc + 1) * 128]
                        .rearrange("(c p) j -> p c j", p=128))

            wo_t = cpool.tile([128, 4 * E], bf16, name="wo_t")
            bq_t = cpool.tile([128, NHC], f32, name="bq_t")
            bk_t = cpool.tile([128, NHC], f32, name="bk_t")
            bv_t = cpool.tile([1, 512], bf16, name="bv_t")
            mask_t = cpool.tile([128, 128], bf16, name="mask_t")

            load_w(wq, wq_t, 0)
            load_w(wk, wk_t, 0)
            nc.sync.dma_start(out=bq_t[:], in_=bq[:, :])
            nc.sync.dma_start(out=bk_t[:], in_=bk[:, :])
            nc.sync.dma_start(out=bv_t[:], in_=bv[:, :])
            load_x(0)
            for hc in range(4):
                load_w(wv, wv_t, hc)
            load_x(1)
            nc.sync.dma_start(out=mask_t[:], in_=mask[:, :])
            load_x(2)
            load_x(3)
            for hc in range(1, 4):
                load_w(wq, wq_t, hc)
                load_w(wk, wk_t, hc)
            nc.sync.dma_start(
                out=wo_t.rearrange("p (c j) -> p c j", c=4),
                in_=wo.rearrange("(c p) j -> p c j", p=128))
            ones1 = cpool.tile([1, 128], bf16, name="ones1")
            nc.vector.memset(ones1[:], 1.0)
            ones_bf = cpool.tile([128, D], bf16, name="ones_bf")
            nc.vector.memset(ones_bf[:], 1.0)

            qt = cpool.tile([128, NHC * S], bf16, name="qt")
            kt = cpool.tile([128, NHC * S], bf16, name="kt")
            vsb = cpool.tile([128, NSC * HPC * VW], bf16, name="vsb")
            aot = cpool.tile([128, NHC * S], bf16, name="aot")
            # ones columns: vsb[(kc, h, D)] = 1
            nc.gpsimd.memset(
                vsb.rearrange("p (c h x) -> p c h x", c=NSC, h=HPC)
                   [:, :, :, D:D + 1], 1.0)

            # ---- projection emission, sliced into ~2-matmul closures ------
            def qk_closures(w_t, b_t, dst, hc, stile):
                state = {}

                def mm(ec):
                    nc.tensor.matmul(
                        state["ps"][:],
                        w_t[:, ec * 512 + hc * 128: ec * 512 + (hc + 1) * 128],
                        xbv[:, ec, stile * 512:(stile + 1) * 512],
                        start=(ec == 0), stop=(ec == 7))

                def c0():
                    state["ps"] = pp_pool.tile(
                        [128, 512], f32, tag="pp",
                        name=f"qk{hc}_{stile}_{w_t.tensor.name}")
                    mm(0); mm(1)

                def c3():
                    mm(6); mm(7)
                    nc.scalar.activation(
                        dst[:, hc * S + stile * 512:
                            hc * S + (stile + 1) * 512],
                        state["ps"][:], AF.Identity,
                        bias=b_t[:, hc:hc + 1], scale=1.0)

                return [c0, lambda: (mm(2), mm(3)), lambda: (mm(4), mm(5)),
                        c3]

            def v_closures(sc):
                # row-major V for all 8 heads, s-chunk sc
                state = {}

                def mm(ec):
                    nc.tensor.matmul(
                        state["vp"][:],
                        xbv[:, ec, sc * 128:(sc + 1) * 128],
                        wv_t[:, ec * 512:(ec + 1) * 512],
                        start=(ec == 0), stop=False)

                def c0():
                    state["vp"] = pp_pool.tile([128, 512], f32, tag="pp",
                                               name=f"vp{sc}")
                    mm(0); mm(1)

                def c3():
                    mm(6); mm(7)
                    nc.tensor.matmul(state["vp"][:], ones1[:, :], bv_t[:, :],
                                     start=False, stop=True)
                    dst = vsb[:, sc * HPC * VW: (sc + 1) * HPC * VW] \
                        .rearrange("p (h x) -> p h x", h=HPC)[:, :, 0:D]
                    nc.vector.tensor_copy(
                        dst, state["vp"].rearrange("p (h x) -> p h x",
                                                   h=HPC))

                return [c0, lambda: (mm(2), mm(3)), lambda: (mm(4), mm(5)),
                        c3]

            def proj_qk_closures(hc):
                gs = []
                for (w_t, b_t, dst) in ((wq_t, bq_t, qt), (wk_t, bk_t, kt)):
                    for stile in range(4):
                        gs.extend(qk_closures(w_t, b_t, dst, hc, stile))
                return gs

            # ---- attention unit -------------------------------------------
            def attention_unit(h, jq, slot=None):
                hc, off = h >> 1, (h & 1) * D
                nkt = 4 * (jq + 1)
                ot = ot_pool.tile([VW, 512], f32, tag="ot", name=f"ot{h}_{jq}")
                for ki in range(nkt):
                    r = ki - 4 * jq
                    wdt = 512 - 128 * r if r >= 0 else 512
                    st = st_pool.tile([128, 512], f32, tag="st",
                                      name=f"st{h}_{jq}_{ki}")
                    pt = pt_pool.tile([128, 512], bf16, tag="pt",
                                      name=f"pt{h}_{jq}_{ki}")
                    nc.tensor.matmul(
                        st[:, 0:wdt],
                        kt[off:off + D,
                           hc * S + ki * 128: hc * S + (ki + 1) * 128],
                        qt[off:off + D,
                           hc * S + jq * 512 + (512 - wdt):
                           hc * S + (jq + 1) * 512],
                        start=True, stop=True)
                    if r >= 0:
                        # boundary 128 cols: exp to staging, then masked mul
                        if wdt > 128:
                            nc.scalar.activation(pt[:, 128:wdt],
                                                 st[:, 128:wdt],
                                                 AF.Exp, scale=0.125)
                        praw = praw_pool.tile([128, 128], bf16, tag="praw",
                                              name=f"praw{h}_{jq}_{ki}")
                        nc.scalar.activation(praw[:], st[:, 0:128],
                                             AF.Exp, scale=0.125)
                        nc.vector.tensor_mul(pt[:, 0:128], praw[:],
                                             mask_t[:])
                    else:
                        nc.scalar.activation(pt[:], st[:],
                                             AF.Exp, scale=0.125)
                    if taps and h == 0 and jq == 0 and ki == 0:
                        nc.sync.dma_start(out=t_pt[:, :], in_=pt[:, :])
                    nc.tensor.matmul(
                        ot[:, (512 - wdt):512],
                        vsb[:, ki * HPC * VW + h * VW:
                            ki * HPC * VW + (h + 1) * VW],
                        pt[:, 0:wdt],
                        start=(ki == 0), stop=(ki == nkt - 1))
                    if slot is not None:
                        slot()
                # normalize: row D of ot = sum_k exp.  Reciprocal on the DVE
                # (fast approx), broadcast across partitions via a contract-1
                # PE matmul with a ones column (baseline-proven pattern).
                # reciprocal_approx_fast (custom DVE uop) emits garbage on
                # HW via this compile path — use the bit-exact InstReciprocal
                # (baseline-proven reading PSUM), writing bf16 directly.
                # The 3.3us InstReciprocal would stall the in-order PE queue
                # at the rb matmul; the caller defers the returned finisher
                # until after the next unit's chunks are emitted.
                den_bf = denb_pool.tile([VW, 512], bf16, tag="denb",
                                        name=f"denb{h}_{jq}")
                with nc.allow_low_precision(reason="bf16 recip feeds PE"):
                    nc.vector.reciprocal(den_bf[D:D + 1, :], ot[D:D + 1, :])

                def finish(h=h, jq=jq, hc=hc, off=off, ot=ot, den_bf=den_bf):
                    rbt = pp_pool.tile([128, 512], f32, tag="pp",
                                       name=f"rb{h}_{jq}")
                    nc.tensor.matmul(rbt[0:D, :], ones_bf[D:D + 1, 0:D],
                                     den_bf[D:D + 1, :], start=True,
                                     stop=True)
                    rbs = rbs_pool.tile([D, 512], bf16, tag="rbs",
                                        name=f"rbs{h}_{jq}")
                    nc.vector.tensor_copy(rbs[:], rbt[0:D, :])
                    if taps and h == 0 and jq == 0:
                        nc.sync.dma_start(out=t_dn[:, :], in_=den_bf[:, :])
                        nc.sync.dma_start(out=t_rb[:, :], in_=rbs[:, :])
                    cols = slice(hc * S + jq * 512, hc * S + (jq + 1) * 512)
                    if off == 0:
                        nc.vector.tensor_mul(aot[0:D, cols], ot[0:D, :],
                                             rbs[:])
                    else:
                        # engines cannot shift partition base; route odd-head
                        # rows to partitions 64..127 via SBUF->SBUF DMA.
                        tmp = tmp_pool.tile([D, 512], bf16, tag="tmp",
                                            name=f"tmp{h}_{jq}")
                        nc.vector.tensor_mul(tmp[:], ot[0:D, :], rbs[:])
                        nc.sync.dma_start(out=aot[D:2 * D, cols], in_=tmp[:])

                return finish

            # ---- schedule -------------------------------------------------
            # pre-phase: QK(0) + V chunks 0..9 emitted inline; the rest are
            # sliced into ~2-matmul closures injected after every attention
            # chunk to keep the PE streaming (max pstate) while the Scalar
            # engine runs the exps.
            for g in proj_qk_closures(0):
                g()
            for sc in range(10):
                for g in v_closures(sc):
                    g()
            fillers = []
            for sc in range(10, 16):
                fillers.extend(v_closures(sc))
            for hc in range(1, 4):
                fillers.extend(proj_qk_closures(hc))

            def slot(n=1):
                for _ in range(n):
                    if fillers:
                        fillers.pop(0)()

            # ---- output projection (one 512-col q-slice at a time) -------
            def outproj_jq(jq):
                for t in range(4 * jq, 4 * jq + 4):
                    for eh in range(2):
                        op = pp_pool.tile([128, 512], f32, tag="pp",
                                          name=f"op{t}_{eh}")
                        for hcc in range(4):
                            nc.tensor.matmul(
                                op[:],
                                aot[:, hcc * S + t * 128:
                                    hcc * S + (t + 1) * 128],
                                wo_t[:, hcc * E + eh * 512:
                                     hcc * E + (eh + 1) * 512],
                                start=(hcc == 0), stop=(hcc == 3))
                        og = og_pool.tile([128, 512], bf16, tag="og",
                                          name=f"og{t}_{eh}")
                        nc.scalar.copy(og[:], op[:])
                        nc.sync.dma_start(
                            out=out[t * 128:(t + 1) * 128,
                                    eh * 512:(eh + 1) * 512],
                            in_=og[:])

            # hc 0-2 run head-major; the last head pair runs jq-major so
            # each out-projection q-slice can be emitted as soon as all 8
            # heads' normalized rows for that slice exist, hiding the
            # out-projection tail under the last attention units.
            units = [(2 * hc + hh, jq) for hc in range(3) for hh in range(2)
                     for jq in range(4)]
            units += [(6 + hh, jq) for jq in range(4) for hh in range(2)]

            done = set()
            emitted = set()

            def after_pop(tag):
                done.add(tag)
                for jq in range(4):
                    if jq not in emitted and (6, jq) in done \
                            and (7, jq) in done:
                        emitted.add(jq)
                        outproj_jq(jq)

            from collections import deque
            pending = deque()
            for (h, jq) in units:
                fin = attention_unit(h, jq)
                pending.append(((h, jq), fin))
                slot(8)
                # defer each unit's norm-finish by 2 units so the 3.3us
                # reciprocal never blocks the in-order PE queue
                if len(pending) > 2:
                    tag, f = pending.popleft()
                    f()
                    after_pop(tag)
            while pending:
                tag, f = pending.popleft()
                f()
                after_pop(tag)
            while fillers:
                fillers.pop(0)()
            assert emitted == {0, 1, 2, 3}

            if taps:
                nc.sync.dma_start(out=t_q[:, :], in_=qt[:, 0:S])
                nc.sync.dma_start(out=t_k[:, :], in_=kt[:, 0:S])
                nc.sync.dma_start(out=t_v[:, :], in_=vsb[:, 0:HPC * VW])
                nc.sync.dma_start(out=t_ao[:, :], in_=aot[:, :])
    nc.compile()
    return nc


def _run(inputs, trace=False, trace_kwargs=None):
    x = np.asarray(inputs["x"], dtype=np.float32)
    wq_w = np.asarray(inputs["wq_w"], dtype=np.float32)
    wq_b = np.asarray(inputs["wq_b"], dtype=np.float32)
    wk_w = np.asarray(inputs["wk_w"], dtype=np.float32)
    wk_b = np.asarray(inputs["wk_b"], dtype=np.float32)
    wv_w = np.asarray(inputs["wv_w"], dtype=np.float32)
    wv_b = np.asarray(inputs["wv_b"], dtype=np.float32)
    wo_w = np.asarray(inputs["wo_w"], dtype=np.float32)
    wo_b = np.asarray(inputs["wo_b"], dtype=np.float32)

    maskh = np.triu(np.ones((128, 128), dtype=np.float32)).astype(BF)

    if "nc" not in _CACHE:
        _CACHE["nc"] = build_nc()
    nc = _CACHE["nc"]

    in_maps = []
    for c in range(NCORES):
        b, hg = c // 2, c % 2
        sl = slice(hg * 512, hg * 512 + 512)
        in_maps.append({
            "xT": np.ascontiguousarray(x[b].T).astype(BF),
            "wq": np.ascontiguousarray(wq_w[:, sl]).astype(BF),
            "wk": np.ascontiguousarray(wk_w[:, sl]).astype(BF),
            "wv": np.ascontiguousarray(wv_w[:, sl]).astype(BF),
            "wo": np.ascontiguousarray(wo_w[sl, :]).astype(BF),
            "bq": np.ascontiguousarray(wq_b[sl].reshape(NHC, 128).T),
            "bk": np.ascontiguousarray(wk_b[sl].reshape(NHC, 128).T),
            "bv": wv_b[sl].reshape(1, 512).astype(BF),
            "mask": maskh,
        })

    kwargs = {}
    if trace:
        kwargs["trace"] = True
        if trace_kwargs:
            kwargs.update(trace_kwargs)
    res = run_bass_kernel_spmd(nc, in_maps, list(range(NCORES)), **kwargs)

    full = np.empty((B, S, E), dtype=np.float32)
    for b in range(B):
        full[b] = (res.results[2 * b]["out"].astype(np.float32)
                   + res.results[2 * b + 1]["out"].astype(np.float32)
                   + wo_b[None, :])
    return full, res


def kernel(**inputs):
    out, _ = _run(inputs, trace=False)
    return out


if __name__ == "__main__":
    rng = np.random.default_rng(0)
    ins = {
        "x": rng.standard_normal((B, S, E), dtype=np.float32),
        "wq_w": rng.standard_normal((E, E), dtype=np.float32) / 32,
        "wq_b": np.zeros(E, np.float32),
        "wk_w": rng.standard_normal((E, E), dtype=np.float32) / 32,
        "wk_b": np.zeros(E, np.float32),
        "wv_w": rng.standard_normal((E, E), dtype=np.float32) / 32,
        "wv_b": np.zeros(E, np.float32),
        "wo_w": rng.standard_normal((E, E), dtype=np.float32) / 32,
        "wo_b": np.zeros(E, np.float32),
    }
    out = kernel(**ins)
    print("ok", out.shape, out.dtype)


# revision 53
# speedup vs baseline: 1.0690x; 1.0053x over previous
"""Causal multi-head self-attention on 8 Trainium2 NeuronCores.

Sharding: (batch, head-group).  Core c owns batch c//2 and heads
(c%2)*8 .. +8 (= 512 of the 1024 feature dims).  Each core:
  - projects Q^T/K^T (dim-major) and V (row-major) for its 512 dims,
  - runs causal attention for its 8 heads over its batch,
  - computes the partial output projection  attn_out @ wo_w[dims, :].
The host sums the 2 partial projections per batch and adds wo_b.

All matmuls run in bf16 (fp32 PSUM accumulation); fp32r was measured
executing as multi-pass fp32_mode=HIGH with heavy power throttling.
Scores are computed transposed (S^T[k, q]); an interleaved ones-column
per head in the V tile makes the P@V matmul emit softmax denominators.
Normalization: DVE InstReciprocal -> ones-column PE matmul broadcast ->
per-head multiply, with each unit's norm-finish deferred by two
attention units so the 3.3us reciprocal never blocks the in-order PE
queue.  Projection matmuls are sliced into 2-matmul closures drained
between attention units; the last head pair runs jq-major so each
512-col out-projection slice is emitted as soon as its rows complete,
hiding the output-projection tail under the final attention units.
"""
import sys
sys.path.insert(0, "/opt/trn_rl_repo")

import numpy as np
import ml_dtypes
import concourse.bass as bass
import concourse.mybir as mybir
from concourse import bacc
from concourse.tile import TileContext
from concourse.bass_utils import run_bass_kernel_spmd

B, S, E = 4, 2048, 1024
H, D = 16, 64
NCORES = 8
HPC = 8                 # heads per core
NHC = 4                 # 128-dim chunks per core (2 heads each)
VW = D + 1              # v cols per head incl ones column
NSC = S // 128          # 16 k/s chunks of 128

f32 = mybir.dt.float32
bf16 = mybir.dt.bfloat16
AF = mybir.ActivationFunctionType
BF = ml_dtypes.bfloat16

_CACHE = {}


def build_nc(taps=False):
    nc = bacc.Bacc("TRN2", target_bir_lowering=False, debug=False)

    xT = nc.declare_dram_parameter("xT", [E, S], bf16, isOutput=False)
    wq = nc.declare_dram_parameter("wq", [E, 512], bf16, isOutput=False)
    wk = nc.declare_dram_parameter("wk", [E, 512], bf16, isOutput=False)
    wv = nc.declare_dram_parameter("wv", [E, 512], bf16, isOutput=False)
    wo = nc.declare_dram_parameter("wo", [512, E], bf16, isOutput=False)
    bq = nc.declare_dram_parameter("bq", [128, NHC], f32, isOutput=False)
    bk = nc.declare_dram_parameter("bk", [128, NHC], f32, isOutput=False)
    bv = nc.declare_dram_parameter("bv", [1, 512], bf16, isOutput=False)
    mask = nc.declare_dram_parameter("mask", [128, 128], bf16, isOutput=False)
    out = nc.declare_dram_parameter("out", [S, E], bf16, isOutput=True)
    if taps:
        t_q = nc.declare_dram_parameter("t_q", [128, S], bf16, isOutput=True)
        t_k = nc.declare_dram_parameter("t_k", [128, S], bf16, isOutput=True)
        t_v = nc.declare_dram_parameter("t_v", [128, HPC * VW], bf16,
                                        isOutput=True)
        t_pt = nc.declare_dram_parameter("t_pt", [128, 512], bf16,
                                         isOutput=True)
        t_dn = nc.declare_dram_parameter("t_dn", [VW, 512], bf16,
                                         isOutput=True)
        t_rb = nc.declare_dram_parameter("t_rb", [D, 512], bf16,
                                         isOutput=True)
        t_ao = nc.declare_dram_parameter("t_ao", [128, NHC * S], bf16,
                                         isOutput=True)

    from contextlib import ExitStack
    with TileContext(nc) as tc:
        with ExitStack() as ctx:
            cpool = ctx.enter_context(tc.tile_pool(name="const", bufs=1))
            pt_pool = ctx.enter_context(tc.tile_pool(name="pt", bufs=4))
            praw_pool = ctx.enter_context(tc.tile_pool(name="praw", bufs=2))
            denb_pool = ctx.enter_context(tc.tile_pool(name="denb", bufs=3))
            rbs_pool = ctx.enter_context(tc.tile_pool(name="rbs", bufs=2))
            tmp_pool = ctx.enter_context(tc.tile_pool(name="tmp", bufs=2))
            og_pool = ctx.enter_context(tc.tile_pool(name="og", bufs=3))
            pp_pool = ctx.enter_context(tc.tile_pool(name="pp", bufs=2,
                                                     space="PSUM"))
            st_pool = ctx.enter_context(tc.tile_pool(name="st", bufs=3,
                                                     space="PSUM"))
            ot_pool = ctx.enter_context(tc.tile_pool(name="ot", bufs=3,
                                                     space="PSUM"))

            # ---- persistent SBUF tensors.  DMA order matters: the first
            # projection group needs wq + x s-block 0, so interleave weight
            # and x loads instead of queueing all 4 MiB of x first.
            xb = cpool.tile([128, 8 * S], bf16, name="xb")
            xbv = xb.rearrange("p (c j) -> p c j", c=8)
            wq_t = cpool.tile([128, 8 * 512], bf16, name="wq_t")
            wk_t = cpool.tile([128, 8 * 512], bf16, name="wk_t")
            wv_t = cpool.tile([128, 8 * 512], bf16, name="wv_t")

            def load_x(sb):
                nc.sync.dma_start(
                    out=xbv[:, :, sb * 512:(sb + 1) * 512],
                    in_=xT[:, sb * 512:(sb + 1) * 512]
                        .rearrange("(c p) j -> p c j", p=128))

            def load_w(w_d, w_s, hc):
                # per-head-chunk weight slice (256KB) so the first
                # projection group isn't gated on a full 1MiB weight DMA
                nc.sync.dma_start(
                    out=w_s.rearrange("p (c j) -> p c j", c=8)
                        [:, :, hc * 128:(hc + 1) * 128],
                    in_=w_d[:, hc * 128:(hc + 1) * 128]
                        .rearrange("(c p) j -> p c j", p=128))

            wo_t = cpool.tile([128, 4 * E], bf16, name="wo_t")
            bq_t = cpool.tile([128, NHC], f32, name="bq_t")
            bk_t = cpool.tile([128, NHC], f32, name="bk_t")
            bv_t = cpool.tile([1, 512], bf16, name="bv_t")
            mask_t = cpool.tile([128, 128], bf16, name="mask_t")

            load_w(wq, wq_t, 0)
            load_w(wk, wk_t, 0)
            nc.sync.dma_start(out=bq_t[:], in_=bq[:, :])
            nc.sync.dma_start(out=bk_t[:], in_=bk[:, :])
            nc.sync.dma_start(out=bv_t[:], in_=bv[:, :])
            load_x(0)
            for hc in range(4):
                load_w(wv, wv_t, hc)
            load_x(1)
            nc.sync.dma_start(out=mask_t[:], in_=mask[:, :])
            load_x(2)
            load_x(3)
            for hc in range(1, 4):
                load_w(wq, wq_t, hc)
                load_w(wk, wk_t, hc)
            nc.sync.dma_start(
                out=wo_t.rearrange("p (c j) -> p c j", c=4),
                in_=wo.rearrange("(c p) j -> p c j", p=128))
            ones1 = cpool.tile([1, 128], bf16, name="ones1")
            nc.vector.memset(ones1[:], 1.0)
            ones_bf = cpool.tile([128, D], bf16, name="ones_bf")
            nc.vector.memset(ones_bf[:], 1.0)

            qt = cpool.tile([128, NHC * S], bf16, name="qt")
            kt = cpool.tile([128, NHC * S], bf16, name="kt")
            vsb = cpool.tile([128, NSC * HPC * VW], bf16, name="vsb")
            aot = cpool.tile([128, NHC * S], bf16, name="aot")
            # ones columns: vsb[(kc, h, D)] = 1
            nc.gpsimd.memset(
                vsb.rearrange("p (c h x) -> p c h x", c=NSC, h=HPC)
                   [:, :, :, D:D + 1], 1.0)

            # ---- projection emission, sliced into ~2-matmul closures ------
            def qk_closures(w_t, b_t, dst, hc, stile):
                state = {}

                def mm(ec):
                    nc.tensor.matmul(
                        state["ps"][:],
                        w_t[:, ec * 512 + hc * 128: ec * 512 + (hc + 1) * 128],
                        xbv[:, ec, stile * 512:(stile + 1) * 512],
                        start=(ec == 0), stop=(ec == 7))

                def c0():
                    state["ps"] = pp_pool.tile(
                        [128, 512], f32, tag="pp",
                        name=f"qk{hc}_{stile}_{w_t.tensor.name}")
                    mm(0); mm(1)

                def c3():
                    mm(6); mm(7)
                    nc.scalar.activation(
                        dst[:, hc * S + stile * 512:
                            hc * S + (stile + 1) * 512],
                        state["ps"][:], AF.Identity,
                        bias=b_t[:, hc:hc + 1], scale=1.0)

                return [c0, lambda: (mm(2), mm(3)), lambda: (mm(4), mm(5)),
                        c3]

            def v_closures(sc):
                # row-major V for all 8 heads, s-chunk sc
                state = {}

                def mm(ec):
                    nc.tensor.matmul(
                        state["vp"][:],
                        xbv[:, ec, sc * 128:(sc + 1) * 128],
                        wv_t[:, ec * 512:(ec + 1) * 512],
                        start=(ec == 0), stop=False)

                def c0():
                    state["vp"] = pp_pool.tile([128, 512], f32, tag="pp",
                                               name=f"vp{sc}")
                    mm(0); mm(1)

                def c3():
                    mm(6); mm(7)
                    nc.tensor.matmul(state["vp"][:], ones1[:, :], bv_t[:, :],
                                     start=False, stop=True)
                    dst = vsb[:, sc * HPC * VW: (sc + 1) * HPC * VW] \
                        .rearrange("p (h x) -> p h x", h=HPC)[:, :, 0:D]
                    nc.vector.tensor_copy(
                        dst, state["vp"].rearrange("p (h x) -> p h x",
                                                   h=HPC))

                return [c0, lambda: (mm(2), mm(3)), lambda: (mm(4), mm(5)),
                        c3]

            def proj_qk_closures(hc):
                gs = []
                for (w_t, b_t, dst) in ((wq_t, bq_t, qt), (wk_t, bk_t, kt)):
                    for stile in range(4):
                        gs.extend(qk_closures(w_t, b_t, dst, hc, stile))
                return gs

            # ---- attention unit -------------------------------------------
            def attention_unit(h, jq, slot=None):
                hc, off = h >> 1, (h & 1) * D
                nkt = 4 * (jq + 1)
                ot = ot_pool.tile([VW, 512], f32, tag="ot", name=f"ot{h}_{jq}")
                for ki in range(nkt):
                    r = ki - 4 * jq
                    wdt = 512 - 128 * r if r >= 0 else 512
                    st = st_pool.tile([128, 512], f32, tag="st",
                                      name=f"st{h}_{jq}_{ki}")
                    pt = pt_pool.tile([128, 512], bf16, tag="pt",
                                      name=f"pt{h}_{jq}_{ki}")
                    nc.tensor.matmul(
                        st[:, 0:wdt],
                        kt[off:off + D,
                           hc * S + ki * 128: hc * S + (ki + 1) * 128],
                        qt[off:off + D,
                           hc * S + jq * 512 + (512 - wdt):
                           hc * S + (jq + 1) * 512],
                        start=True, stop=True)
                    if r >= 0:
                        # boundary 128 cols: exp to staging, then masked mul
                        if wdt > 128:
                            nc.scalar.activation(pt[:, 128:wdt],
                                                 st[:, 128:wdt],
                                                 AF.Exp, scale=0.125)
                        praw = praw_pool.tile([128, 128], bf16, tag="praw",
                                              name=f"praw{h}_{jq}_{ki}")
                        nc.scalar.activation(praw[:], st[:, 0:128],
                                             AF.Exp, scale=0.125)
                        nc.vector.tensor_mul(pt[:, 0:128], praw[:],
                                             mask_t[:])
                    else:
                        nc.scalar.activation(pt[:], st[:],
                                             AF.Exp, scale=0.125)
                    if taps and h == 0 and jq == 0 and ki == 0:
                        nc.sync.dma_start(out=t_pt[:, :], in_=pt[:, :])
                    nc.tensor.matmul(
                        ot[:, (512 - wdt):512],
                        vsb[:, ki * HPC * VW + h * VW:
                            ki * HPC * VW + (h + 1) * VW],
                        pt[:, 0:wdt],
                        start=(ki == 0), stop=(ki == nkt - 1))
                    if slot is not None:
                        slot()
                # normalize: row D of ot = sum_k exp.  Reciprocal on the DVE
                # (fast approx), broadcast across partitions via a contract-1
                # PE matmul with a ones column (baseline-proven pattern).
                # reciprocal_approx_fast (custom DVE uop) emits garbage on
                # HW via this compile path — use the bit-exact InstReciprocal
                # (baseline-proven reading PSUM), writing bf16 directly.
                # The 3.3us InstReciprocal would stall the in-order PE queue
                # at the rb matmul; the caller defers the returned finisher
                # until after the next unit's chunks are emitted.
                den_bf = denb_pool.tile([VW, 512], bf16, tag="denb",
                                        name=f"denb{h}_{jq}")
                with nc.allow_low_precision(reason="bf16 recip feeds PE"):
                    nc.vector.reciprocal(den_bf[D:D + 1, :], ot[D:D + 1, :])

                def finish(h=h, jq=jq, hc=hc, off=off, ot=ot, den_bf=den_bf):
                    rbt = pp_pool.tile([128, 512], f32, tag="pp",
                                       name=f"rb{h}_{jq}")
                    nc.tensor.matmul(rbt[0:D, :], ones_bf[D:D + 1, 0:D],
                                     den_bf[D:D + 1, :], start=True,
                                     stop=True)
                    rbs = rbs_pool.tile([D, 512], bf16, tag="rbs",
                                        name=f"rbs{h}_{jq}")
                    nc.vector.tensor_copy(rbs[:], rbt[0:D, :])
                    if taps and h == 0 and jq == 0:
                        nc.sync.dma_start(out=t_dn[:, :], in_=den_bf[:, :])
                        nc.sync.dma_start(out=t_rb[:, :], in_=rbs[:, :])
                    cols = slice(hc * S + jq * 512, hc * S + (jq + 1) * 512)
                    if off == 0:
                        nc.vector.tensor_mul(aot[0:D, cols], ot[0:D, :],
                                             rbs[:])
                    else:
                        # engines cannot shift partition base; route odd-head
                        # rows to partitions 64..127 via SBUF->SBUF DMA.
                        tmp = tmp_pool.tile([D, 512], bf16, tag="tmp",
                                            name=f"tmp{h}_{jq}")
                        nc.vector.tensor_mul(tmp[:], ot[0:D, :], rbs[:])
                        nc.sync.dma_start(out=aot[D:2 * D, cols], in_=tmp[:])

                return finish

            # ---- schedule -------------------------------------------------
            # pre-phase: QK(0) + V chunks 0..9 emitted inline; the rest are
            # sliced into ~2-matmul closures injected after every attention
            # chunk to keep the PE streaming (max pstate) while the Scalar
            # engine runs the exps.
            for g in proj_qk_closures(0):
                g()
            for sc in range(10):
                for g in v_closures(sc):
                    g()
            fillers = []
            for sc in range(10, 16):
                fillers.extend(v_closures(sc))
            for hc in range(1, 4):
                fillers.extend(proj_qk_closures(hc))

            def slot(n=1):
                for _ in range(n):
                    if fillers:
                        fillers.pop(0)()

            # ---- output projection (one 512-col q-slice at a time) -------
            def outproj_jq(jq):
                for t in range(4 * jq, 4 * jq + 4):
                    for eh in range(2):
                        op = pp_pool.tile([128, 512], f32, tag="pp",
                                          name=f"op{t}_{eh}")
                        for hcc in range(4):
                            nc.tensor.matmul(
                                op[:],
                                aot[:, hcc * S + t * 128:
                                    hcc * S + (t + 1) * 128],
                                wo_t[:, hcc * E + eh * 512:
                                     hcc * E + (eh + 1) * 512],
                                start=(hcc == 0), stop=(hcc == 3))
                        og = og_pool.tile([128, 512], bf16, tag="og",
                                          name=f"og{t}_{eh}")
                        nc.scalar.copy(og[:], op[:])
                        nc.sync.dma_start(
                            out=out[t * 128:(t + 1) * 128,
                                    eh * 512:(eh + 1) * 512],
                            in_=og[:])

            # hc 0-2 run head-major; the last head pair runs jq-major so
            # each out-projection q-slice can be emitted as soon as all 8
            # heads' normalized rows for that slice exist, hiding the
            # out-projection tail under the last attention units.
            # first half head-major (heads 0-3, fillers active); back half
            # jq-major across heads 4-7 so each out-projection q-slice can
            # fire as soon as its jq completes, becoming the late-phase PE
            # filler once the projection closures are exhausted.
            units = [(2 * hc + hh, jq) for hc in range(2) for hh in range(2)
                     for jq in range(4)]
            units += [(4 + hh, jq) for jq in range(4) for hh in range(4)]

            done = set()
            emitted = set()

            def after_pop(tag):
                done.add(tag)
                for jq in range(4):
                    if jq not in emitted \
                            and all((h, jq) in done for h in range(8)):
                        emitted.add(jq)
                        outproj_jq(jq)

            from collections import deque
            pending = deque()
            for (h, jq) in units:
                fin = attention_unit(h, jq)
                pending.append(((h, jq), fin))
                slot(8)
                # defer each unit's norm-finish by 2 units so the 3.3us
                # reciprocal never blocks the in-order PE queue
                if len(pending) > 2:
                    tag, f = pending.popleft()
                    f()
                    after_pop(tag)
            while pending:
                tag, f = pending.popleft()
                f()
                after_pop(tag)
            while fillers:
                fillers.pop(0)()
            assert emitted == {0, 1, 2, 3}

            if taps:
                nc.sync.dma_start(out=t_q[:, :], in_=qt[:, 0:S])
                nc.sync.dma_start(out=t_k[:, :], in_=kt[:, 0:S])
                nc.sync.dma_start(out=t_v[:, :], in_=vsb[:, 0:HPC * VW])
                nc.sync.dma_start(out=t_ao[:, :], in_=aot[:, :])
    nc.compile()
    return nc


def _run(inputs, trace=False, trace_kwargs=None):
    x = np.asarray(inputs["x"], dtype=np.float32)
    wq_w = np.asarray(inputs["wq_w"], dtype=np.float32)
    wq_b = np.asarray(inputs["wq_b"], dtype=np.float32)
    wk_w = np.asarray(inputs["wk_w"], dtype=np.float32)
    wk_b = np.asarray(inputs["wk_b"], dtype=np.float32)
    wv_w = np.asarray(inputs["wv_w"], dtype=np.float32)
    wv_b = np.asarray(inputs["wv_b"], dtype=np.float32)
    wo_w = np.asarray(inputs["wo_w"], dtype=np.float32)
    wo_b = np.asarray(inputs["wo_b"], dtype=np.float32)

    maskh = np.triu(np.ones((128, 128), dtype=np.float32)).astype(BF)

    if "nc" not in _CACHE:
        _CACHE["nc"] = build_nc()
    nc = _CACHE["nc"]

    in_maps = []
    for c in range(NCORES):
        b, hg = c // 2, c % 2
        sl = slice(hg * 512, hg * 512 + 512)
        in_maps.append({
            "xT": np.ascontiguousarray(x[b].T).astype(BF),
            "wq": np.ascontiguousarray(wq_w[:, sl]).astype(BF),
            "wk": np.ascontiguousarray(wk_w[:, sl]).astype(BF),
            "wv": np.ascontiguousarray(wv_w[:, sl]).astype(BF),
            "wo": np.ascontiguousarray(wo_w[sl, :]).astype(BF),
            "bq": np.ascontiguousarray(wq_b[sl].reshape(NHC, 128).T),
            "bk": np.ascontiguousarray(wk_b[sl].reshape(NHC, 128).T),
            "bv": wv_b[sl].reshape(1, 512).astype(BF),
            "mask": maskh,
        })

    kwargs = {}
    if trace:
        kwargs["trace"] = True
        if trace_kwargs:
            kwargs.update(trace_kwargs)
    res = run_bass_kernel_spmd(nc, in_maps, list(range(NCORES)), **kwargs)

    full = np.empty((B, S, E), dtype=np.float32)
    for b in range(B):
        full[b] = (res.results[2 * b]["out"].astype(np.float32)
                   + res.results[2 * b + 1]["out"].astype(np.float32)
                   + wo_b[None, :])
    return full, res


def kernel(**inputs):
    out, _ = _run(inputs, trace=False)
    return out


if __name__ == "__main__":
    rng = np.random.default_rng(0)
    ins = {
        "x": rng.standard_normal((B, S, E), dtype=np.float32),
        "wq_w": rng.standard_normal((E, E), dtype=np.float32) / 32,
        "wq_b": np.zeros(E, np.float32),
        "wk_w": rng.standard_normal((E, E), dtype=np.float32) / 32,
        "wk_b": np.zeros(E, np.float32),
        "wv_w": rng.standard_normal((E, E), dtype=np.float32) / 32,
        "wv_b": np.zeros(E, np.float32),
        "wo_w": rng.standard_normal((E, E), dtype=np.float32) / 32,
        "wo_b": np.zeros(E, np.float32),
    }
    out = kernel(**ins)
    print("ok", out.shape, out.dtype)
